# revision 12
# baseline (speedup 1.0000x reference)
"""Single-head attention (no 1/sqrt(d) scaling) for Trainium2, 8 NeuronCores.

Problem: x [8, 2048, 768], W [2304, 768], b [2304]
    qkv = x @ W.T + b ; q,k,v = split(qkv)
    out = softmax(q @ k.T) @ v            -> [8, 2048, 768] fp32

Sharding: data-parallel over batch, one batch element per core.

Weight folding (host-side, exact): softmax over keys m is invariant to
per-query constants, so with gm = Wq.T @ Wk and a = Wk.T @ bq,
    S'[n,m] = (x @ gm + a) @ x.T
satisfies softmax(S') == softmax(q @ k.T) row-for-row (the x_n.bk and
bq.bk terms cancel; verified to 3e-15 in fp64). This deletes the entire
k projection (1/3 of the QKV GEMM work) from the device: the kernel keeps
x^T resident in SBUF (where k^T used to live) and projects a single
z = x @ gm + a strip per n-slice (same cost as the old q strip).

All matmuls run in fp32r (full PE rate at 512-wide moving operands,
~1.5e-4 rel rounding — bf16 anywhere upstream of the softmax would blow
up the logit error).

Phase A: v projection only (v = x @ Wv.T + bv, natural layout, resident),
after loading all of x^T into 24 resident [128,512] tiles. A run of dummy
warmup matmuls on a memset tile keeps the PE busy (and the HAM clock at
2.4 GHz) while the first loads land.

Phase B per 512-wide n-slice:
    z strip = (x @ gm + a)^T  (6 accumulating matmuls per h-chunk)
    S'^T[m,n] = x z           (lhsT = resident x^T blocks)
    P = exp(S'^T)             (ACT; no max subtraction, |logits| << 88)
    U^T += v_m^T @ P          (6 PSUM banks, accumulated over 16 m-chunks)
    acc += P                  (DVE; per-partition denominator partials)
    r = partition_all_reduce(acc)   (GPSIMD, once per slice — off the PE)
    out^T slice = U^T * approx(1/r) (DVE fast reciprocal + multiply)
The last slice skips normalization: U banks are copied out (DVE/ACT) and
stored with raw acc; the host divides during the gather. This removes the
allreduce+reciprocal+multiply chain from the device-side tail.

DMA queues: scalar (ACT HWDGE) carries x slices 0-1 + half the stores;
sync (SP HWDGE) x slices 2-3 + the other half; gpsimd (SWDGE) carries
wv/gm and the per-slice partition_all_reduce, and issues no stores so its
expensive DGE drain runs long before the kernel tail.
"""

import contextlib

import numpy as np

import concourse.bacc as bacc
import concourse.bass_isa as bass_isa
import concourse.mybir as mybir
import concourse.tile as tile
from concourse.bass_utils import run_bass_kernel_spmd

F32 = mybir.dt.float32
F32R = mybir.dt.float32r
AF = mybir.ActivationFunctionType
ALU = mybir.AluOpType

B, N, H = 8, 2048, 768
P = 128
ND = H // P      # 6 h-chunks
NM = N // P      # 16 m-chunks
SL = 512         # n-slice width (fp32 moving-operand max / one PSUM bank)
NSL = N // SL    # 4 n-slices
TAIL_SLICE = NSL - 1


def build_nc(loop_iters=None, split=1, nm_eff=NM,
             fast_recip=True, host_tail=True, warmup=28):
    nc = bacc.Bacc("TRN2", target_bir_lowering=False, debug=False)

    xT = nc.dram_tensor("xT", [H, N], F32R, kind="ExternalInput")
    wvT = nc.dram_tensor("wvT", [H, H], F32R, kind="ExternalInput")
    gm = nc.dram_tensor("gm", [H, H], F32R, kind="ExternalInput")
    bcol = nc.dram_tensor("bcol", [P, ND], F32, kind="ExternalInput")
    bvrep = nc.dram_tensor("bvrep", [P, H], F32, kind="ExternalInput")
    out = nc.dram_tensor("out", [H, N], F32, kind="ExternalOutput")  # transposed
    if host_tail:
        ulast = nc.dram_tensor("ulast", [H, SL], F32, kind="ExternalOutput")
        racc = nc.dram_tensor("racc", [P, SL], F32, kind="ExternalOutput")

    def mm_group(psum, lhs_list, rhs_slicer, split=1):
        width = psum.shape[-1]
        hw = width // split
        n = len(lhs_list)
        steps = [(c, h) for c in range(n) for h in range(split)]
        for idx, (c, h) in enumerate(steps):
            lo = h * hw
            nc.tensor.matmul(
                psum[:, lo : lo + hw], lhs_list[c], rhs_slicer(c, lo, hw),
                start=(idx == 0), stop=(idx == len(steps) - 1),
            )

    with tile.TileContext(nc) as tc:
        with (
            tc.tile_pool(name="dram", bufs=1, space="DRAM") as dram,
            tc.tile_pool(name="const", bufs=1) as const,
            tc.tile_pool(name="keep", bufs=1) as keep,
            tc.For_i(0, loop_iters, 1) if loop_iters else contextlib.nullcontext(),
        ):
            bcol_sb = const.tile([P, ND], F32)

            # resident: all of x^T (24 tiles) + v (16 tiles)
            xr = [
                [keep.tile([P, SL], F32R, name=f"x{c}_{s}") for s in range(NSL)]
                for c in range(ND)
            ]
            vsb = [keep.tile([P, H], F32R, name=f"v{ni}") for ni in range(NM)]

            with tc.tile_pool(name="xw_pool", bufs=1) as xw:
                HH = H // 2
                gmt = [
                    [xw.tile([P, HH], F32R, name=f"gm{c}_{h}") for h in range(2)]
                    for c in range(ND)
                ]

                def gslice(c, hc):
                    half, col = divmod(hc * P, HH)
                    return gmt[c][half][:, col : col + P]

                xwa = tc.alloc_tile_pool(name="xwa_pool", bufs=1)
                wv = [xwa.tile([P, H], F32R, name=f"wv{c}") for c in range(ND)]
                bvb = xwa.tile([P, H], F32, name="bvb")
                warm_f32 = xwa.tile([P, SL], F32, name="warm_sb")
                warm_sb = warm_f32[:].bitcast(F32R)

                # ---- startup DMA schedule -----------------------------------
                # scalar: bcol, x slices 0+1, bvb; sync: x slices 2+3;
                # gpsimd (SWDGE): wv then gm. No gpsimd compute before the
                # issues — a blocked queue head would delay everything behind
                # it (partition_broadcast cost 14us of wv issue delay in v4).
                nc.gpsimd.memset(warm_f32[:], 0.0)
                nc.scalar.dma_start(bcol_sb[:], bcol.ap())
                for c in range(ND):
                    nc.gpsimd.dma_start(
                        wv[c][:], wvT.ap()[c * P : (c + 1) * P, :]
                    )
                for s in range(NSL):
                    eng = nc.scalar if s in (0, 1) else nc.sync
                    for c in range(ND):
                        eng.dma_start(
                            xr[c][s][:],
                            xT.ap()[c * P : (c + 1) * P, s * SL : (s + 1) * SL],
                        )
                nc.scalar.dma_start(bvb[:], bvrep.ap())
                for h in range(2):
                    for c in range(ND):
                        nc.gpsimd.dma_start(
                            gmt[c][h][:],
                            gm.ap()[c * P : (c + 1) * P, h * HH : (h + 1) * HH],
                        )

                # ---- Phase A: v projection ----------------------------------
                with (
                    tc.tile_pool(name="vps", bufs=2, space="PSUM") as vps,
                    tc.tile_pool(name="wmps", bufs=1, space="PSUM") as wmps,
                ):
                    if warmup:
                        wps = wmps.tile([P, SL], F32, name="warm_ps")
                        for _ in range(warmup):
                            nc.tensor.matmul(
                                wps[:], warm_sb[:, 0:P], warm_sb,
                                start=True, stop=True,
                            )

                    # consume slices in their DMA-landing order: s0/s2 stream
                    # on scalar/sync in parallel, then s1/s3
                    for s in (0, 2, 1, 3):
                      for blk in range(NSL):
                        ni = s * NSL + blk
                        lsl = slice(blk * P, (blk + 1) * P)
                        pa = vps.tile([P, SL], F32, name="pa", tag="pa")
                        pb = vps.tile([P, H - SL], F32, name="pb", tag="pb")
                        mm_group(
                            pa, [xr[c][s][:, lsl] for c in range(ND)],
                            lambda c, lo, w: wv[c][:, lo : lo + w],
                            split=split,
                        )
                        mm_group(
                            pb, [xr[c][s][:, lsl] for c in range(ND)],
                            lambda c, lo, w: wv[c][:, SL + lo : SL + lo + w],
                        )
                        nc.vector.tensor_tensor(
                            vsb[ni][:, 0:SL], pa[:], bvb[:, 0:SL], op=ALU.add
                        )
                        nc.vector.tensor_tensor(
                            vsb[ni][:, SL:H], pb[:], bvb[:, SL:H], op=ALU.add
                        )

                xwa.release()

                # ---- Phase B: attention (software-pipelined m-loop) ----
                with (
                    tc.tile_pool(name="zsb_pool", bufs=2) as zsb_pool,
                    tc.tile_pool(name="p_pool", bufs=4) as p_pool,
                    tc.tile_pool(name="u_ps", bufs=1, space="PSUM") as u_ps,
                    tc.tile_pool(name="sps", bufs=2, space="PSUM") as sps,
                    tc.tile_pool(name="usb_pool", bufs=1) as usb_pool,
                    tc.tile_pool(name="misc", bufs=1) as misc,
                ):
                    for ns in range(NSL):
                        tail = host_tail and ns == TAIL_SLICE
                        # project this slice's z strip (transposed layout)
                        zbuf = []
                        for hc in range(ND):
                            ps = sps.tile([P, SL], F32, name="s_ps", tag="s")
                            mm_group(
                                ps, [gslice(c, hc) for c in range(ND)],
                                lambda c, lo, w, _ns=ns: xr[c][_ns][:, lo : lo + w],
                                split=split,
                            )
                            zc = zsb_pool.tile([P, SL], F32R, name=f"zsb{hc}", tag=f"zsb{hc}")
                            nc.scalar.activation(
                                zc[:], ps[:], AF.Identity, bias=bcol_sb[:, hc : hc + 1]
                            )
                            zbuf.append(zc)
                        us = [
                            u_ps.tile([P, SL], F32, name=f"u{c}", tag=f"u{c}")
                            for c in range(ND)
                        ]
                        acc = misc.tile([P, SL], F32, name="acc", tag="acc", bufs=2)

                        p_sbs = [None] * NM
                        for mi in range(nm_eff + 1):
                            if mi < nm_eff:
                                s, blk = divmod(mi, NSL)
                                msl = slice(blk * P, (blk + 1) * P)
                                s_ps = sps.tile([P, SL], F32, name="s_ps", tag="s")
                                mm_group(
                                    s_ps, [xr[c][s][:, msl] for c in range(ND)],
                                    lambda c, lo, w: zbuf[c][:, lo : lo + w],
                                    split=split,
                                )
                                p_sb = p_pool.tile([P, SL], F32R, name="p_sb", tag="p")
                                nc.scalar.activation(p_sb[:], s_ps[:], AF.Exp)
                                p_sbs[mi] = p_sb
                            if mi >= 1:
                                j = mi - 1
                                pj = p_sbs[j]
                                if j == 0:
                                    nc.vector.tensor_copy(acc[:], pj[:])
                                else:
                                    nc.vector.tensor_tensor(
                                        acc[:], pj[:], acc[:], op=ALU.add
                                    )
                                for c in range(ND):
                                    nc.tensor.matmul(
                                        us[c][:],
                                        vsb[j][:, c * P : (c + 1) * P],
                                        pj[:],
                                        start=(j == 0),
                                        stop=(j == nm_eff - 1),
                                    )
                                p_sbs[j] = None

                        # copy-then-scale eviction: raw copies (DVE/ACT split)
                        # free the U PSUM banks right after the last matmul, so
                        # the next slice's U accumulation never waits on the
                        # allreduce -> reciprocal -> multiply chain
                        if tail:
                            nc.sync.dma_start(racc.ap(), acc[:])
                        u_sbs = []
                        for c in range(ND):
                            u_sb = usb_pool.tile(
                                [P, SL], F32, name=f"usb{c}", tag=f"usb{c}"
                            )
                            if c % 2 == 0:
                                nc.vector.tensor_copy(u_sb[:], us[c][:])
                            else:
                                nc.scalar.copy(u_sb[:], us[c][:])
                            u_sbs.append(u_sb)
                        if tail:
                            # unnormalized exit; host divides at the gather
                            for c in range(ND):
                                store_eng = nc.sync if c % 2 == 0 else nc.scalar
                                store_eng.dma_start(
                                    ulast.ap()[c * P : (c + 1) * P, :], u_sbs[c][:]
                                )
                        else:
                            rall = misc.tile([P, SL], F32, name="rall", tag="rall")
                            nc.gpsimd.partition_all_reduce(
                                rall[:], acc[:], P, bass_isa.ReduceOp.add
                            )
                            rinv = misc.tile([P, SL], F32, name="rinv", tag="rinv")
                            if fast_recip:
                                nc.vector.reciprocal_approx_fast(rinv[:], rall[:])
                            else:
                                nc.vector.reciprocal(rinv[:], rall[:])
                            for c in range(ND):
                                nc.vector.tensor_tensor(
                                    u_sbs[c][:], u_sbs[c][:], rinv[:], op=ALU.mult
                                )
                                store_eng = nc.sync if c % 2 == 0 else nc.scalar
                                store_eng.dma_start(
                                    out.ap()[c * P : (c + 1) * P, ns * SL : (ns + 1) * SL],
                                    u_sbs[c][:],
                                )

    nc.compile()
    return nc


_NC = None


def make_in_maps(x, W, b):
    x = np.ascontiguousarray(x, dtype=np.float32)
    W = np.asarray(W, dtype=np.float32)
    b = np.asarray(b, dtype=np.float32)
    Wq, Wk, Wv = W[:H], W[H : 2 * H], W[2 * H :]
    bq = b[:H]
    gm_host = np.ascontiguousarray(Wq.T @ Wk)                  # [768, 768]
    a = Wk.T @ bq                                              # [768]
    wvT = np.ascontiguousarray(Wv.T)                           # [768, 768]
    bcol = np.ascontiguousarray(a.reshape(ND, P).T)            # [128, 6]
    bvrep = np.ascontiguousarray(
        np.broadcast_to(b[2 * H :].reshape(1, H), (P, H))
    )
    return [
        {
            "xT": np.ascontiguousarray(x[i].T),                # [768, 2048]
            "wvT": wvT,
            "gm": gm_host,
            "bcol": bcol,
            "bvrep": bvrep,
        }
        for i in range(B)
    ]


def kernel(x: np.ndarray, W: np.ndarray, b: np.ndarray) -> np.ndarray:
    global _NC
    if _NC is None:
        _NC = build_nc()

    in_maps = make_in_maps(x, W, b)
    res = run_bass_kernel_spmd(_NC, in_maps, core_ids=list(range(B)))
    outs = []
    for i in range(B):
        oT = np.array(res.results[i]["out"])                   # [768, 2048]
        if "ulast" in res.results[i]:
            ul = np.asarray(res.results[i]["ulast"], dtype=np.float64)
            ra = np.asarray(res.results[i]["racc"], dtype=np.float64)
            r = ra.sum(axis=0)                                 # [512]
            lo = TAIL_SLICE * SL
            oT[:, lo : lo + SL] = (ul / r[None, :]).astype(np.float32)
        outs.append(np.ascontiguousarray(oT.T))
    return np.stack(outs, axis=0)


# revision 16
# speedup vs baseline: 1.0003x; 1.0003x over previous
"""Single-head attention (no 1/sqrt(d) scaling) for Trainium2, 8 NeuronCores.

Problem: x [8, 2048, 768], W [2304, 768], b [2304]
    qkv = x @ W.T + b ; q,k,v = split(qkv)
    out = softmax(q @ k.T) @ v            -> [8, 2048, 768] fp32

Sharding: data-parallel over batch, one batch element per core.

Weight folding (host-side, exact): softmax over keys m is invariant to
per-query constants, so with gm = Wq.T @ Wk and a = Wk.T @ bq,
    S'[n,m] = (x @ gm + a) @ x.T
satisfies softmax(S') == softmax(q @ k.T) row-for-row (the x_n.bk and
bq.bk terms cancel; verified to 3e-15 in fp64). This deletes the entire
k projection (1/3 of the QKV GEMM work) from the device: the kernel keeps
x^T resident in SBUF (where k^T used to live) and projects a single
z = x @ gm + a strip per n-slice (same cost as the old q strip).

All matmuls run in fp32r (full PE rate at 512-wide moving operands,
~1.5e-4 rel rounding — bf16 anywhere upstream of the softmax would blow
up the logit error).

Phase A: v projection only (v = x @ Wv.T + bv, natural layout, resident),
after loading all of x^T into 24 resident [128,512] tiles. A run of dummy
warmup matmuls on a memset tile keeps the PE busy (and the HAM clock at
2.4 GHz) while the first loads land.

Phase B per 512-wide n-slice:
    z strip = (x @ gm + a)^T  (6 accumulating matmuls per h-chunk)
    S'^T[m,n] = x z           (lhsT = resident x^T blocks)
    P = exp(S'^T)             (ACT; no max subtraction, |logits| << 88)
    U^T += v_m^T @ P          (6 PSUM banks, accumulated over 16 m-chunks)
    acc += P                  (DVE; per-partition denominator partials)
    r = partition_all_reduce(acc)   (GPSIMD, once per slice — off the PE)
    out^T slice = U^T * approx(1/r) (DVE fast reciprocal + multiply)
The last slice skips normalization: U banks are copied out (DVE/ACT) and
stored with raw acc; the host divides during the gather. This removes the
allreduce+reciprocal+multiply chain from the device-side tail.

DMA queues: scalar (ACT HWDGE) carries x slices 0-1 + half the stores;
sync (SP HWDGE) x slices 2-3 + the other half; gpsimd (SWDGE) carries
wv/gm and the per-slice partition_all_reduce, and issues no stores so its
expensive DGE drain runs long before the kernel tail.
"""

import contextlib

import numpy as np

import concourse.bacc as bacc
import concourse.bass_isa as bass_isa
import concourse.mybir as mybir
import concourse.tile as tile
from concourse.bass_utils import run_bass_kernel_spmd

F32 = mybir.dt.float32
F32R = mybir.dt.float32r
AF = mybir.ActivationFunctionType
ALU = mybir.AluOpType

B, N, H = 8, 2048, 768
P = 128
ND = H // P      # 6 h-chunks
NM = N // P      # 16 m-chunks
SL = 512         # n-slice width (fp32 moving-operand max / one PSUM bank)
NSL = N // SL    # 4 n-slices
TAIL_SLICE = NSL - 1


def build_nc(loop_iters=None, split=1, nm_eff=NM,
             fast_recip=True, host_tail=True, warmup=28):
    nc = bacc.Bacc("TRN2", target_bir_lowering=False, debug=False)

    xT = nc.dram_tensor("xT", [H, N], F32R, kind="ExternalInput")
    wvT = nc.dram_tensor("wvT", [H, H], F32R, kind="ExternalInput")
    gm = nc.dram_tensor("gm", [H, H], F32R, kind="ExternalInput")
    bcol = nc.dram_tensor("bcol", [P, ND], F32, kind="ExternalInput")
    bvrep = nc.dram_tensor("bvrep", [P, H], F32, kind="ExternalInput")
    out = nc.dram_tensor("out", [H, N], F32, kind="ExternalOutput")  # transposed
    if host_tail:
        ulast = nc.dram_tensor("ulast", [H, SL], F32, kind="ExternalOutput")
        racc = nc.dram_tensor("racc", [P, SL], F32, kind="ExternalOutput")

    def mm_group(psum, lhs_list, rhs_slicer, split=1):
        width = psum.shape[-1]
        hw = width // split
        n = len(lhs_list)
        steps = [(c, h) for c in range(n) for h in range(split)]
        for idx, (c, h) in enumerate(steps):
            lo = h * hw
            nc.tensor.matmul(
                psum[:, lo : lo + hw], lhs_list[c], rhs_slicer(c, lo, hw),
                start=(idx == 0), stop=(idx == len(steps) - 1),
            )

    with tile.TileContext(nc) as tc:
        with (
            tc.tile_pool(name="dram", bufs=1, space="DRAM") as dram,
            tc.tile_pool(name="const", bufs=1) as const,
            tc.tile_pool(name="keep", bufs=1) as keep,
            tc.For_i(0, loop_iters, 1) if loop_iters else contextlib.nullcontext(),
        ):
            bcol_sb = const.tile([P, ND], F32)

            # resident: all of x^T (24 tiles) + v (16 tiles) + z strips (24)
            xr = [
                [keep.tile([P, SL], F32R, name=f"x{c}_{s}") for s in range(NSL)]
                for c in range(ND)
            ]
            vsb = [keep.tile([P, H], F32R, name=f"v{ni}") for ni in range(NM)]
            zall = [
                [keep.tile([P, SL], F32R, name=f"z{hc}_{s}") for hc in range(ND)]
                for s in range(NSL)
            ]

            with tc.tile_pool(name="xw_pool", bufs=1) as xw:
                HH = H // 2
                gmt = [
                    [xw.tile([P, HH], F32R, name=f"gm{c}_{h}") for h in range(2)]
                    for c in range(ND)
                ]

                def gslice(c, hc):
                    half, col = divmod(hc * P, HH)
                    return gmt[c][half][:, col : col + P]

                xwa = tc.alloc_tile_pool(name="xwa_pool", bufs=1)
                wv = [xwa.tile([P, H], F32R, name=f"wv{c}") for c in range(ND)]
                bvb = xwa.tile([P, H], F32, name="bvb")
                warm_f32 = xwa.tile([P, SL], F32, name="warm_sb")
                warm_sb = warm_f32[:].bitcast(F32R)

                # ---- startup DMA schedule -----------------------------------
                # scalar: bcol, x slices 0+1, bvb; sync: x slices 2+3;
                # gpsimd (SWDGE): wv then gm. No gpsimd compute before the
                # issues — a blocked queue head would delay everything behind
                # it (partition_broadcast cost 14us of wv issue delay in v4).
                nc.gpsimd.memset(warm_f32[:], 0.0)
                nc.scalar.dma_start(bcol_sb[:], bcol.ap())
                for c in range(ND):
                    nc.gpsimd.dma_start(
                        wv[c][:], wvT.ap()[c * P : (c + 1) * P, :]
                    )
                for s in range(NSL):
                    eng = nc.scalar if s in (0, 1) else nc.sync
                    for c in range(ND):
                        eng.dma_start(
                            xr[c][s][:],
                            xT.ap()[c * P : (c + 1) * P, s * SL : (s + 1) * SL],
                        )
                nc.scalar.dma_start(bvb[:], bvrep.ap())
                for h in range(2):
                    for c in range(ND):
                        nc.gpsimd.dma_start(
                            gmt[c][h][:],
                            gm.ap()[c * P : (c + 1) * P, h * HH : (h + 1) * HH],
                        )

                # ---- Phase A: v projection + all z strips -------------------
                # v projections run first (paced by the x/wv streams); the z
                # projections follow as pure PE work once gm lands, filling
                # what used to be phase-B time while the DMAs finish
                with (
                    tc.tile_pool(name="vps", bufs=2, space="PSUM") as vps,
                    tc.tile_pool(name="zps", bufs=2, space="PSUM") as zps,
                    tc.tile_pool(name="wmps", bufs=1, space="PSUM") as wmps,
                ):
                    if warmup:
                        wps = wmps.tile([P, SL], F32, name="warm_ps")
                        for _ in range(warmup):
                            nc.tensor.matmul(
                                wps[:], warm_sb[:, 0:P], warm_sb,
                                start=True, stop=True,
                            )

                    # consume slices in their DMA-landing order: s0/s2 stream
                    # on scalar/sync in parallel, then s1/s3
                    for s in (0, 2, 1, 3):
                      for blk in range(NSL):
                        ni = s * NSL + blk
                        lsl = slice(blk * P, (blk + 1) * P)
                        pa = vps.tile([P, SL], F32, name="pa", tag="pa")
                        pb = vps.tile([P, H - SL], F32, name="pb", tag="pb")
                        mm_group(
                            pa, [xr[c][s][:, lsl] for c in range(ND)],
                            lambda c, lo, w: wv[c][:, lo : lo + w],
                            split=split,
                        )
                        mm_group(
                            pb, [xr[c][s][:, lsl] for c in range(ND)],
                            lambda c, lo, w: wv[c][:, SL + lo : SL + lo + w],
                        )
                        nc.vector.tensor_tensor(
                            vsb[ni][:, 0:SL], pa[:], bvb[:, 0:SL], op=ALU.add
                        )
                        nc.vector.tensor_tensor(
                            vsb[ni][:, SL:H], pb[:], bvb[:, SL:H], op=ALU.add
                        )

                    for zs in range(NSL):
                        for hc in range(ND):
                            ps = zps.tile([P, SL], F32, name="z_ps", tag="z")
                            mm_group(
                                ps, [gslice(c, hc) for c in range(ND)],
                                lambda c, lo, w, _s=zs: xr[c][_s][:, lo : lo + w],
                                split=split,
                            )
                            nc.scalar.activation(
                                zall[zs][hc][:], ps[:], AF.Identity,
                                bias=bcol_sb[:, hc : hc + 1],
                            )

                xwa.release()

                # ---- Phase B: attention (software-pipelined m-loop) ----
                with (
                    tc.tile_pool(name="p_pool", bufs=4) as p_pool,
                    tc.tile_pool(name="u_ps", bufs=1, space="PSUM") as u_ps,
                    tc.tile_pool(name="sps", bufs=2, space="PSUM") as sps,
                    tc.tile_pool(name="usb_pool", bufs=1) as usb_pool,
                    tc.tile_pool(name="misc", bufs=1) as misc,
                ):
                    for ns in range(NSL):
                        tail = host_tail and ns == TAIL_SLICE
                        zbuf = zall[ns]
                        us = [
                            u_ps.tile([P, SL], F32, name=f"u{c}", tag=f"u{c}")
                            for c in range(ND)
                        ]
                        acc = misc.tile([P, SL], F32, name="acc", tag="acc", bufs=2)

                        p_sbs = [None] * NM
                        for mi in range(nm_eff + 1):
                            if mi < nm_eff:
                                s, blk = divmod(mi, NSL)
                                msl = slice(blk * P, (blk + 1) * P)
                                s_ps = sps.tile([P, SL], F32, name="s_ps", tag="s")
                                mm_group(
                                    s_ps, [xr[c][s][:, msl] for c in range(ND)],
                                    lambda c, lo, w: zbuf[c][:, lo : lo + w],
                                    split=split,
                                )
                                p_sb = p_pool.tile([P, SL], F32R, name="p_sb", tag="p")
                                nc.scalar.activation(p_sb[:], s_ps[:], AF.Exp)
                                p_sbs[mi] = p_sb
                            if mi >= 1:
                                j = mi - 1
                                pj = p_sbs[j]
                                if j == 0:
                                    nc.vector.tensor_copy(acc[:], pj[:])
                                else:
                                    nc.vector.tensor_tensor(
                                        acc[:], pj[:], acc[:], op=ALU.add
                                    )
                                for c in range(ND):
                                    nc.tensor.matmul(
                                        us[c][:],
                                        vsb[j][:, c * P : (c + 1) * P],
                                        pj[:],
                                        start=(j == 0),
                                        stop=(j == nm_eff - 1),
                                    )
                                p_sbs[j] = None

                        # copy-then-scale eviction: raw copies (DVE/ACT split)
                        # free the U PSUM banks right after the last matmul, so
                        # the next slice's U accumulation never waits on the
                        # allreduce -> reciprocal -> multiply chain
                        if tail:
                            nc.sync.dma_start(racc.ap(), acc[:])
                        u_sbs = []
                        for c in range(ND):
                            u_sb = usb_pool.tile(
                                [P, SL], F32, name=f"usb{c}", tag=f"usb{c}"
                            )
                            if c % 2 == 0:
                                nc.vector.tensor_copy(u_sb[:], us[c][:])
                            else:
                                nc.scalar.copy(u_sb[:], us[c][:])
                            u_sbs.append(u_sb)
                        if tail:
                            # unnormalized exit; host divides at the gather
                            for c in range(ND):
                                store_eng = nc.sync if c % 2 == 0 else nc.scalar
                                store_eng.dma_start(
                                    ulast.ap()[c * P : (c + 1) * P, :], u_sbs[c][:]
                                )
                        else:
                            rall = misc.tile([P, SL], F32, name="rall", tag="rall")
                            nc.gpsimd.partition_all_reduce(
                                rall[:], acc[:], P, bass_isa.ReduceOp.add
                            )
                            rinv = misc.tile([P, SL], F32, name="rinv", tag="rinv")
                            if fast_recip:
                                nc.vector.reciprocal_approx_fast(rinv[:], rall[:])
                            else:
                                nc.vector.reciprocal(rinv[:], rall[:])
                            for c in range(ND):
                                nc.vector.tensor_tensor(
                                    u_sbs[c][:], u_sbs[c][:], rinv[:], op=ALU.mult
                                )
                                store_eng = nc.sync if c % 2 == 0 else nc.scalar
                                store_eng.dma_start(
                                    out.ap()[c * P : (c + 1) * P, ns * SL : (ns + 1) * SL],
                                    u_sbs[c][:],
                                )

    nc.compile()
    return nc


_NC = None


def make_in_maps(x, W, b):
    x = np.ascontiguousarray(x, dtype=np.float32)
    W = np.asarray(W, dtype=np.float32)
    b = np.asarray(b, dtype=np.float32)
    Wq, Wk, Wv = W[:H], W[H : 2 * H], W[2 * H :]
    bq = b[:H]
    gm_host = np.ascontiguousarray(Wq.T @ Wk)                  # [768, 768]
    a = Wk.T @ bq                                              # [768]
    wvT = np.ascontiguousarray(Wv.T)                           # [768, 768]
    bcol = np.ascontiguousarray(a.reshape(ND, P).T)            # [128, 6]
    bvrep = np.ascontiguousarray(
        np.broadcast_to(b[2 * H :].reshape(1, H), (P, H))
    )
    return [
        {
            "xT": np.ascontiguousarray(x[i].T),                # [768, 2048]
            "wvT": wvT,
            "gm": gm_host,
            "bcol": bcol,
            "bvrep": bvrep,
        }
        for i in range(B)
    ]


def kernel(x: np.ndarray, W: np.ndarray, b: np.ndarray) -> np.ndarray:
    global _NC
    if _NC is None:
        _NC = build_nc()

    in_maps = make_in_maps(x, W, b)
    res = run_bass_kernel_spmd(_NC, in_maps, core_ids=list(range(B)))
    outs = []
    for i in range(B):
        oT = np.array(res.results[i]["out"])                   # [768, 2048]
        if "ulast" in res.results[i]:
            ul = np.asarray(res.results[i]["ulast"], dtype=np.float64)
            ra = np.asarray(res.results[i]["racc"], dtype=np.float64)
            r = ra.sum(axis=0)                                 # [512]
            lo = TAIL_SLICE * SL
            oT[:, lo : lo + SL] = (ul / r[None, :]).astype(np.float32)
        outs.append(np.ascontiguousarray(oT.T))
    return np.stack(outs, axis=0)


# revision 20
# speedup vs baseline: 1.0367x; 1.0364x over previous
"""Single-head attention (no 1/sqrt(d) scaling) for Trainium2, 8 NeuronCores.

Problem: x [8, 2048, 768], W [2304, 768], b [2304]
    qkv = x @ W.T + b ; q,k,v = split(qkv)
    out = softmax(q @ k.T) @ v            -> [8, 2048, 768] fp32

Sharding: data-parallel over batch, one batch element per core.

Weight folding (host-side, exact): softmax over keys m is invariant to
per-query constants, so with gm = Wq.T @ Wk and a = Wk.T @ bq,
    S'[n,m] = (x @ gm + a) @ x.T
satisfies softmax(S') == softmax(q @ k.T) row-for-row (the x_n.bk and
bq.bk terms cancel; verified to 3e-15 in fp64). This deletes the entire
k projection (1/3 of the QKV GEMM work) from the device: the kernel keeps
x^T resident in SBUF (where k^T used to live) and projects a single
z = x @ gm + a strip per n-slice (same cost as the old q strip).

All matmuls run in fp32r (full PE rate at 512-wide moving operands,
~1.5e-4 rel rounding — bf16 anywhere upstream of the softmax would blow
up the logit error).

Phase A: v projection only (v = x @ Wv.T + bv, natural layout, resident),
after loading all of x^T into 24 resident [128,512] tiles. A run of dummy
warmup matmuls on a memset tile keeps the PE busy (and the HAM clock at
2.4 GHz) while the first loads land.

Phase B per 512-wide n-slice:
    z strip = (x @ gm + a)^T  (6 accumulating matmuls per h-chunk)
    S'^T[m,n] = x z           (lhsT = resident x^T blocks)
    P = exp(S'^T)             (ACT; no max subtraction, |logits| << 88)
    U^T += v_m^T @ P          (6 PSUM banks, accumulated over 16 m-chunks)
    acc += P                  (DVE; per-partition denominator partials)
    r = partition_all_reduce(acc)   (GPSIMD, once per slice — off the PE)
    out^T slice = U^T * approx(1/r) (DVE fast reciprocal + multiply)
The last slice skips normalization: U banks are copied out (DVE/ACT) and
stored with raw acc; the host divides during the gather. This removes the
allreduce+reciprocal+multiply chain from the device-side tail.

DMA queues: scalar (ACT HWDGE) carries x slices 0-1 + half the stores;
sync (SP HWDGE) x slices 2-3 + the other half; gpsimd (SWDGE) carries
wv/gm and the per-slice partition_all_reduce, and issues no stores so its
expensive DGE drain runs long before the kernel tail.
"""

import contextlib

import numpy as np

import concourse.bacc as bacc
import concourse.bass_isa as bass_isa
import concourse.mybir as mybir
import concourse.tile as tile
from concourse.bass_utils import run_bass_kernel_spmd

F32 = mybir.dt.float32
F32R = mybir.dt.float32r
AF = mybir.ActivationFunctionType
ALU = mybir.AluOpType

B, N, H = 8, 2048, 768
P = 128
ND = H // P      # 6 h-chunks
NM = N // P      # 16 m-chunks
SL = 512         # n-slice width (fp32 moving-operand max / one PSUM bank)
NSL = N // SL    # 4 n-slices
TAIL_SLICE = NSL - 1


def build_nc(loop_iters=None, split=1, nm_eff=NM,
             fast_recip=True, host_tail=True, warmup=25):
    nc = bacc.Bacc("TRN2", target_bir_lowering=False, debug=False)

    xT = nc.dram_tensor("xT", [H, N], F32R, kind="ExternalInput")
    wvT = nc.dram_tensor("wvT", [H, H], F32R, kind="ExternalInput")
    gm = nc.dram_tensor("gm", [H, H], F32R, kind="ExternalInput")
    bcol = nc.dram_tensor("bcol", [P, ND], F32, kind="ExternalInput")
    bvrep = nc.dram_tensor("bvrep", [P, H], F32, kind="ExternalInput")
    out = nc.dram_tensor("out", [H, N], F32, kind="ExternalOutput")  # transposed
    if host_tail:
        ulast = nc.dram_tensor("ulast", [H, SL], F32, kind="ExternalOutput")
        racc = nc.dram_tensor("racc", [P, SL], F32, kind="ExternalOutput")

    def mm_group(psum, lhs_list, rhs_slicer, split=1):
        width = psum.shape[-1]
        hw = width // split
        n = len(lhs_list)
        steps = [(c, h) for c in range(n) for h in range(split)]
        for idx, (c, h) in enumerate(steps):
            lo = h * hw
            nc.tensor.matmul(
                psum[:, lo : lo + hw], lhs_list[c], rhs_slicer(c, lo, hw),
                start=(idx == 0), stop=(idx == len(steps) - 1),
            )

    with tile.TileContext(nc) as tc:
        with (
            tc.tile_pool(name="dram", bufs=1, space="DRAM") as dram,
            tc.tile_pool(name="const", bufs=1) as const,
            tc.tile_pool(name="keep", bufs=1) as keep,
            tc.For_i(0, loop_iters, 1) if loop_iters else contextlib.nullcontext(),
        ):
            bcol_sb = const.tile([P, ND], F32)

            # resident: all of x^T (24 tiles) + v (16 tiles) + z strips (24)
            xr = [
                [keep.tile([P, SL], F32R, name=f"x{c}_{s}") for s in range(NSL)]
                for c in range(ND)
            ]
            vsb = [keep.tile([P, H], F32R, name=f"v{ni}") for ni in range(NM)]
            zall = [
                [keep.tile([P, SL], F32R, name=f"z{hc}_{s}") for hc in range(ND)]
                for s in range(NSL)
            ]

            with tc.tile_pool(name="xw_pool", bufs=1) as xw:
                HH = H // 2
                gmt = [
                    [xw.tile([P, HH], F32R, name=f"gm{c}_{h}") for h in range(2)]
                    for c in range(ND)
                ]

                def gslice(c, hc):
                    half, col = divmod(hc * P, HH)
                    return gmt[c][half][:, col : col + P]

                xwa = tc.alloc_tile_pool(name="xwa_pool", bufs=1)
                wv = [xwa.tile([P, H], F32R, name=f"wv{c}") for c in range(ND)]
                bvb = xwa.tile([P, H], F32, name="bvb")
                warm_f32 = xwa.tile([P, SL], F32, name="warm_sb")
                warm_sb = warm_f32[:].bitcast(F32R)

                # ---- startup DMA schedule -----------------------------------
                # scalar: bcol, x slices 0+1, bvb; sync: x slices 2+3, wv;
                # gpsimd (SWDGE): gm only, so the z projections (the first
                # real PE work after warmup) unlock as early as possible.
                nc.gpsimd.memset(warm_f32[:], 0.0)
                nc.scalar.dma_start(bcol_sb[:], bcol.ap())
                for h in range(2):
                    for c in range(ND):
                        nc.gpsimd.dma_start(
                            gmt[c][h][:],
                            gm.ap()[c * P : (c + 1) * P, h * HH : (h + 1) * HH],
                        )
                for s in range(NSL):
                    eng = nc.scalar if s in (0, 1) else nc.sync
                    for c in range(ND):
                        eng.dma_start(
                            xr[c][s][:],
                            xT.ap()[c * P : (c + 1) * P, s * SL : (s + 1) * SL],
                        )
                for c in range(ND):
                    nc.sync.dma_start(
                        wv[c][:], wvT.ap()[c * P : (c + 1) * P, :]
                    )
                nc.scalar.dma_start(bvb[:], bvrep.ap())

                # ---- Phase A: v projection + all z strips -------------------
                # v projections run first (paced by the x/wv streams); the z
                # projections follow as pure PE work once gm lands, filling
                # what used to be phase-B time while the DMAs finish
                with (
                    tc.tile_pool(name="vps", bufs=2, space="PSUM") as vps,
                    tc.tile_pool(name="zps", bufs=2, space="PSUM") as zps,
                    tc.tile_pool(name="wmps", bufs=1, space="PSUM") as wmps,
                ):
                    if warmup:
                        wps = wmps.tile([P, SL], F32, name="warm_ps")
                        for _ in range(warmup):
                            nc.tensor.matmul(
                                wps[:], warm_sb[:, 0:P], warm_sb,
                                start=True, stop=True,
                            )

                    # z projections first (gm lands earliest on the SWDGE
                    # queue), then v projections (wv streams on sync behind
                    # the x slices). Slice order matches DMA landing order:
                    # s0 (scalar) / s2 (sync) stream first, then s1 / s3.
                    for zs in (0, 2, 1, 3):
                        for hc in range(ND):
                            ps = zps.tile([P, SL], F32, name="z_ps", tag="z")
                            mm_group(
                                ps, [gslice(c, hc) for c in range(ND)],
                                lambda c, lo, w, _s=zs: xr[c][_s][:, lo : lo + w],
                                split=split,
                            )
                            nc.scalar.activation(
                                zall[zs][hc][:], ps[:], AF.Identity,
                                bias=bcol_sb[:, hc : hc + 1],
                            )

                    for s in (0, 2, 1, 3):
                      for blk in range(NSL):
                        ni = s * NSL + blk
                        lsl = slice(blk * P, (blk + 1) * P)
                        pa = vps.tile([P, SL], F32, name="pa", tag="pa")
                        pb = vps.tile([P, H - SL], F32, name="pb", tag="pb")
                        mm_group(
                            pa, [xr[c][s][:, lsl] for c in range(ND)],
                            lambda c, lo, w: wv[c][:, lo : lo + w],
                            split=split,
                        )
                        mm_group(
                            pb, [xr[c][s][:, lsl] for c in range(ND)],
                            lambda c, lo, w: wv[c][:, SL + lo : SL + lo + w],
                        )
                        nc.vector.tensor_tensor(
                            vsb[ni][:, 0:SL], pa[:], bvb[:, 0:SL], op=ALU.add
                        )
                        nc.vector.tensor_tensor(
                            vsb[ni][:, SL:H], pb[:], bvb[:, SL:H], op=ALU.add
                        )

                xwa.release()

                # ---- Phase B: attention (software-pipelined m-loop) ----
                with (
                    tc.tile_pool(name="p_pool", bufs=5) as p_pool,
                    tc.tile_pool(name="u_ps", bufs=1, space="PSUM") as u_ps,
                    tc.tile_pool(name="sps", bufs=2, space="PSUM") as sps,
                    tc.tile_pool(name="usb_pool", bufs=1) as usb_pool,
                    tc.tile_pool(name="misc", bufs=1) as misc,
                ):
                    for ns in range(NSL):
                        tail = host_tail and ns == TAIL_SLICE
                        zbuf = zall[ns]
                        us = [
                            u_ps.tile([P, SL], F32, name=f"u{c}", tag=f"u{c}")
                            for c in range(ND)
                        ]
                        acc = misc.tile([P, SL], F32, name="acc", tag="acc", bufs=2)

                        p_sbs = [None] * NM
                        for mi in range(nm_eff + 1):
                            if mi < nm_eff:
                                s, blk = divmod(mi, NSL)
                                msl = slice(blk * P, (blk + 1) * P)
                                s_ps = sps.tile([P, SL], F32, name="s_ps", tag="s")
                                mm_group(
                                    s_ps, [xr[c][s][:, msl] for c in range(ND)],
                                    lambda c, lo, w: zbuf[c][:, lo : lo + w],
                                    split=split,
                                )
                                p_sb = p_pool.tile([P, SL], F32R, name="p_sb", tag="p")
                                nc.scalar.activation(p_sb[:], s_ps[:], AF.Exp)
                                p_sbs[mi] = p_sb
                            if mi >= 1:
                                j = mi - 1
                                pj = p_sbs[j]
                                if j == 0:
                                    nc.vector.tensor_copy(acc[:], pj[:])
                                else:
                                    nc.vector.tensor_tensor(
                                        acc[:], pj[:], acc[:], op=ALU.add
                                    )
                                for c in range(ND):
                                    nc.tensor.matmul(
                                        us[c][:],
                                        vsb[j][:, c * P : (c + 1) * P],
                                        pj[:],
                                        start=(j == 0),
                                        stop=(j == nm_eff - 1),
                                    )
                                p_sbs[j] = None

                        # copy-then-scale eviction: raw copies (DVE/ACT split)
                        # free the U PSUM banks right after the last matmul, so
                        # the next slice's U accumulation never waits on the
                        # allreduce -> reciprocal -> multiply chain
                        # copy-then-scale eviction: raw DVE copies free the U
                        # PSUM banks right after the last matmul, and ACT does
                        # nothing here so the next slice's first exp is not
                        # delayed behind eviction work
                        if tail:
                            nc.sync.dma_start(racc.ap(), acc[:])
                        u_sbs = []
                        for c in range(ND):
                            u_sb = usb_pool.tile(
                                [P, SL], F32, name=f"usb{c}", tag=f"usb{c}"
                            )
                            nc.vector.tensor_copy(u_sb[:], us[c][:])
                            u_sbs.append(u_sb)
                        if tail:
                            # unnormalized exit; host divides at the gather
                            for c in range(ND):
                                store_eng = nc.sync if c % 2 == 0 else nc.scalar
                                store_eng.dma_start(
                                    ulast.ap()[c * P : (c + 1) * P, :], u_sbs[c][:]
                                )
                        else:
                            rall = misc.tile([P, SL], F32, name="rall", tag="rall")
                            nc.gpsimd.partition_all_reduce(
                                rall[:], acc[:], P, bass_isa.ReduceOp.add
                            )
                            rinv = misc.tile([P, SL], F32, name="rinv", tag="rinv")
                            if fast_recip:
                                nc.vector.reciprocal_approx_fast(rinv[:], rall[:])
                            else:
                                nc.vector.reciprocal(rinv[:], rall[:])
                            for c in range(ND):
                                nc.vector.tensor_tensor(
                                    u_sbs[c][:], u_sbs[c][:], rinv[:], op=ALU.mult
                                )
                                nc.sync.dma_start(
                                    out.ap()[c * P : (c + 1) * P, ns * SL : (ns + 1) * SL],
                                    u_sbs[c][:],
                                )

    nc.compile()
    return nc


_NC = None


def make_in_maps(x, W, b):
    x = np.ascontiguousarray(x, dtype=np.float32)
    W = np.asarray(W, dtype=np.float32)
    b = np.asarray(b, dtype=np.float32)
    Wq, Wk, Wv = W[:H], W[H : 2 * H], W[2 * H :]
    bq = b[:H]
    gm_host = np.ascontiguousarray(Wq.T @ Wk)                  # [768, 768]
    a = Wk.T @ bq                                              # [768]
    wvT = np.ascontiguousarray(Wv.T)                           # [768, 768]
    bcol = np.ascontiguousarray(a.reshape(ND, P).T)            # [128, 6]
    bvrep = np.ascontiguousarray(
        np.broadcast_to(b[2 * H :].reshape(1, H), (P, H))
    )
    return [
        {
            "xT": np.ascontiguousarray(x[i].T),                # [768, 2048]
            "wvT": wvT,
            "gm": gm_host,
            "bcol": bcol,
            "bvrep": bvrep,
        }
        for i in range(B)
    ]


def kernel(x: np.ndarray, W: np.ndarray, b: np.ndarray) -> np.ndarray:
    global _NC
    if _NC is None:
        _NC = build_nc()

    in_maps = make_in_maps(x, W, b)
    res = run_bass_kernel_spmd(_NC, in_maps, core_ids=list(range(B)))
    outs = []
    for i in range(B):
        oT = np.array(res.results[i]["out"])                   # [768, 2048]
        if "ulast" in res.results[i]:
            ul = np.asarray(res.results[i]["ulast"], dtype=np.float64)
            ra = np.asarray(res.results[i]["racc"], dtype=np.float64)
            r = ra.sum(axis=0)                                 # [512]
            lo = TAIL_SLICE * SL
            oT[:, lo : lo + SL] = (ul / r[None, :]).astype(np.float32)
        outs.append(np.ascontiguousarray(oT.T))
    return np.stack(outs, axis=0)


# revision 24
# speedup vs baseline: 1.0422x; 1.0053x over previous
"""Single-head attention (no 1/sqrt(d) scaling) for Trainium2, 8 NeuronCores.

Problem: x [8, 2048, 768], W [2304, 768], b [2304]
    qkv = x @ W.T + b ; q,k,v = split(qkv)
    out = softmax(q @ k.T) @ v            -> [8, 2048, 768] fp32

Sharding: data-parallel over batch, one batch element per core.

Weight folding (host-side, exact): softmax over keys m is invariant to
per-query constants, so with gm = Wq.T @ Wk and a = Wk.T @ bq,
    S'[n,m] = (x @ gm + a) @ x.T
satisfies softmax(S') == softmax(q @ k.T) row-for-row (the x_n.bk and
bq.bk terms cancel; verified to 3e-15 in fp64). This deletes the entire
k projection (1/3 of the QKV GEMM work) from the device: the kernel keeps
x^T resident in SBUF (where k^T used to live) and projects a single
z = x @ gm + a strip per n-slice (same cost as the old q strip).

All matmuls run in fp32r (full PE rate at 512-wide moving operands,
~1.5e-4 rel rounding — bf16 anywhere upstream of the softmax would blow
up the logit error).

Phase A: v projection only (v = x @ Wv.T + bv, natural layout, resident),
after loading all of x^T into 24 resident [128,512] tiles. A run of dummy
warmup matmuls on a memset tile keeps the PE busy (and the HAM clock at
2.4 GHz) while the first loads land.

Phase B per 512-wide n-slice:
    z strip = (x @ gm + a)^T  (6 accumulating matmuls per h-chunk)
    S'^T[m,n] = x z           (lhsT = resident x^T blocks)
    P = exp(S'^T)             (ACT; no max subtraction, |logits| << 88)
    U^T += v_m^T @ P          (6 PSUM banks, accumulated over 16 m-chunks)
    acc += P                  (DVE; per-partition denominator partials)
    r = partition_all_reduce(acc)   (GPSIMD, once per slice — off the PE)
    out^T slice = U^T * approx(1/r) (DVE fast reciprocal + multiply)
The last slice skips normalization: U banks are copied out (DVE/ACT) and
stored with raw acc; the host divides during the gather. This removes the
allreduce+reciprocal+multiply chain from the device-side tail.

DMA queues: scalar (ACT HWDGE) carries x slices 0-1 + half the stores;
sync (SP HWDGE) x slices 2-3 + the other half; gpsimd (SWDGE) carries
wv/gm and the per-slice partition_all_reduce, and issues no stores so its
expensive DGE drain runs long before the kernel tail.
"""

import contextlib

import numpy as np

import concourse.bacc as bacc
import concourse.bass_isa as bass_isa
import concourse.mybir as mybir
import concourse.tile as tile
from concourse.bass_utils import run_bass_kernel_spmd

F32 = mybir.dt.float32
F32R = mybir.dt.float32r
AF = mybir.ActivationFunctionType
ALU = mybir.AluOpType

B, N, H = 8, 2048, 768
P = 128
ND = H // P      # 6 h-chunks
NM = N // P      # 16 m-chunks
SL = 512         # n-slice width (fp32 moving-operand max / one PSUM bank)
NSL = N // SL    # 4 n-slices
TAIL_SLICE = NSL - 1


def build_nc(loop_iters=None, split=1, nm_eff=NM,
             fast_recip=True, host_tail=True, warmup=25):
    nc = bacc.Bacc("TRN2", target_bir_lowering=False, debug=False)

    xT = nc.dram_tensor("xT", [H, N], F32R, kind="ExternalInput")
    wvT = nc.dram_tensor("wvT", [H, H], F32R, kind="ExternalInput")
    gm = nc.dram_tensor("gm", [H, H], F32R, kind="ExternalInput")
    bcol = nc.dram_tensor("bcol", [P, ND], F32, kind="ExternalInput")
    bvrep = nc.dram_tensor("bvrep", [P, H], F32, kind="ExternalInput")
    out = nc.dram_tensor("out", [H, N], F32, kind="ExternalOutput")  # transposed
    if host_tail:
        ulast = nc.dram_tensor("ulast", [H, SL], F32, kind="ExternalOutput")
        racc = nc.dram_tensor("racc", [P, SL], F32, kind="ExternalOutput")

    def mm_group(psum, lhs_list, rhs_slicer, split=1):
        width = psum.shape[-1]
        hw = width // split
        n = len(lhs_list)
        steps = [(c, h) for c in range(n) for h in range(split)]
        for idx, (c, h) in enumerate(steps):
            lo = h * hw
            nc.tensor.matmul(
                psum[:, lo : lo + hw], lhs_list[c], rhs_slicer(c, lo, hw),
                start=(idx == 0), stop=(idx == len(steps) - 1),
            )

    with tile.TileContext(nc) as tc:
        with (
            tc.tile_pool(name="dram", bufs=1, space="DRAM") as dram,
            tc.tile_pool(name="const", bufs=1) as const,
            tc.tile_pool(name="keep", bufs=1) as keep,
            tc.For_i(0, loop_iters, 1) if loop_iters else contextlib.nullcontext(),
        ):
            bcol_sb = const.tile([P, ND], F32)

            # resident: all of x^T (24 tiles) + v (16 tiles) + z strips (24)
            xr = [
                [keep.tile([P, SL], F32R, name=f"x{c}_{s}") for s in range(NSL)]
                for c in range(ND)
            ]
            vsb = [keep.tile([P, H], F32R, name=f"v{ni}") for ni in range(NM)]
            zall = [
                [keep.tile([P, SL], F32R, name=f"z{hc}_{s}") for hc in range(ND)]
                for s in range(NSL)
            ]

            with tc.tile_pool(name="xw_pool", bufs=1) as xw:
                HH = H // 2
                gmt = [
                    [xw.tile([P, HH], F32R, name=f"gm{c}_{h}") for h in range(2)]
                    for c in range(ND)
                ]

                def gslice(c, hc):
                    half, col = divmod(hc * P, HH)
                    return gmt[c][half][:, col : col + P]

                xwa = tc.alloc_tile_pool(name="xwa_pool", bufs=1)
                wv = [xwa.tile([P, H], F32R, name=f"wv{c}") for c in range(ND)]
                bvb = xwa.tile([P, H], F32, name="bvb")
                warm_f32 = xwa.tile([P, SL], F32, name="warm_sb")
                warm_sb = warm_f32[:].bitcast(F32R)

                # ---- startup DMA schedule -----------------------------------
                # Landing order matched to the interleaved z/v emission below:
                # scalar: bcol, x s0, wv, bvb; sync: x s2, s1, s3;
                # gpsimd (SWDGE): gm only, so the z projections (the first
                # real PE work after warmup) unlock as early as possible.
                nc.gpsimd.memset(warm_f32[:], 0.0)
                nc.scalar.dma_start(bcol_sb[:], bcol.ap())
                for h in range(2):
                    for c in range(ND):
                        nc.gpsimd.dma_start(
                            gmt[c][h][:],
                            gm.ap()[c * P : (c + 1) * P, h * HH : (h + 1) * HH],
                        )

                def load_x(s, eng):
                    for c in range(ND):
                        eng.dma_start(
                            xr[c][s][:],
                            xT.ap()[c * P : (c + 1) * P, s * SL : (s + 1) * SL],
                        )

                load_x(0, nc.scalar)
                load_x(2, nc.sync)
                for c in range(ND):
                    nc.scalar.dma_start(
                        wv[c][:], wvT.ap()[c * P : (c + 1) * P, :]
                    )
                load_x(1, nc.sync)
                nc.scalar.dma_start(bvb[:], bvrep.ap())
                load_x(3, nc.sync)

                # ---- Phase A: v projection + all z strips -------------------
                # v projections run first (paced by the x/wv streams); the z
                # projections follow as pure PE work once gm lands, filling
                # what used to be phase-B time while the DMAs finish
                with (
                    tc.tile_pool(name="vps", bufs=2, space="PSUM") as vps,
                    tc.tile_pool(name="zps", bufs=2, space="PSUM") as zps,
                    tc.tile_pool(name="wmps", bufs=1, space="PSUM") as wmps,
                ):
                    wps = wmps.tile([P, SL], F32, name="warm_ps")
                    if warmup:
                        for _ in range(warmup):
                            nc.tensor.matmul(
                                wps[:], warm_sb[:, 0:P], warm_sb,
                                start=True, stop=True,
                            )

                    # Interleaved z/v emission: z slices whose x lands early
                    # run first; v slices slot into the windows where later x
                    # tiles are still streaming (PE is in-order, so emission
                    # order must track data-landing order). Warm-fill matmuls
                    # between the first z slice's groups bridge the gm/x
                    # trickle so the HAM clock never drops.
                    def z_proj(zs, fill=0):
                        for hc in range(ND):
                            ps = zps.tile([P, SL], F32, name="z_ps", tag="z")
                            mm_group(
                                ps, [gslice(c, hc) for c in range(ND)],
                                lambda c, lo, w, _s=zs: xr[c][_s][:, lo : lo + w],
                                split=split,
                            )
                            nc.scalar.activation(
                                zall[zs][hc][:], ps[:], AF.Identity,
                                bias=bcol_sb[:, hc : hc + 1],
                            )
                            for _ in range(fill):
                                nc.tensor.matmul(
                                    wps[:], warm_sb[:, 0:P], warm_sb,
                                    start=True, stop=True,
                                )

                    def v_proj(s):
                        for blk in range(NSL):
                            ni = s * NSL + blk
                            lsl = slice(blk * P, (blk + 1) * P)
                            pa = vps.tile([P, SL], F32, name="pa", tag="pa")
                            pb = vps.tile([P, H - SL], F32, name="pb", tag="pb")
                            mm_group(
                                pa, [xr[c][s][:, lsl] for c in range(ND)],
                                lambda c, lo, w: wv[c][:, lo : lo + w],
                                split=split,
                            )
                            mm_group(
                                pb, [xr[c][s][:, lsl] for c in range(ND)],
                                lambda c, lo, w: wv[c][:, SL + lo : SL + lo + w],
                            )
                            nc.vector.tensor_tensor(
                                vsb[ni][:, 0:SL], pa[:], bvb[:, 0:SL], op=ALU.add
                            )
                            nc.vector.tensor_tensor(
                                vsb[ni][:, SL:H], pb[:], bvb[:, SL:H], op=ALU.add
                            )

                    z_proj(0, fill=1)
                    z_proj(2)
                    v_proj(0)
                    v_proj(2)
                    z_proj(1)
                    v_proj(1)
                    z_proj(3)
                    v_proj(3)

                xwa.release()

                # ---- Phase B: attention (software-pipelined m-loop) ----
                with (
                    tc.tile_pool(name="p_pool", bufs=5) as p_pool,
                    tc.tile_pool(name="u_ps", bufs=1, space="PSUM") as u_ps,
                    tc.tile_pool(name="sps", bufs=2, space="PSUM") as sps,
                    tc.tile_pool(name="usb_pool", bufs=1) as usb_pool,
                    tc.tile_pool(name="misc", bufs=1) as misc,
                ):
                    for ns in range(NSL):
                        tail = host_tail and ns == TAIL_SLICE
                        zbuf = zall[ns]
                        us = [
                            u_ps.tile([P, SL], F32, name=f"u{c}", tag=f"u{c}")
                            for c in range(ND)
                        ]
                        acc = misc.tile([P, SL], F32, name="acc", tag="acc", bufs=2)

                        p_sbs = [None] * NM
                        for mi in range(nm_eff + 1):
                            if mi < nm_eff:
                                s, blk = divmod(mi, NSL)
                                msl = slice(blk * P, (blk + 1) * P)
                                s_ps = sps.tile([P, SL], F32, name="s_ps", tag="s")
                                mm_group(
                                    s_ps, [xr[c][s][:, msl] for c in range(ND)],
                                    lambda c, lo, w: zbuf[c][:, lo : lo + w],
                                    split=split,
                                )
                                p_sb = p_pool.tile([P, SL], F32R, name="p_sb", tag="p")
                                nc.scalar.activation(p_sb[:], s_ps[:], AF.Exp)
                                p_sbs[mi] = p_sb
                            if mi >= 1:
                                j = mi - 1
                                pj = p_sbs[j]
                                if j == 0:
                                    nc.vector.tensor_copy(acc[:], pj[:])
                                else:
                                    nc.vector.tensor_tensor(
                                        acc[:], pj[:], acc[:], op=ALU.add
                                    )
                                for c in range(ND):
                                    nc.tensor.matmul(
                                        us[c][:],
                                        vsb[j][:, c * P : (c + 1) * P],
                                        pj[:],
                                        start=(j == 0),
                                        stop=(j == nm_eff - 1),
                                    )
                                p_sbs[j] = None

                        # copy-then-scale eviction: raw copies (DVE/ACT split)
                        # free the U PSUM banks right after the last matmul, so
                        # the next slice's U accumulation never waits on the
                        # allreduce -> reciprocal -> multiply chain
                        # copy-then-scale eviction: raw DVE copies free the U
                        # PSUM banks right after the last matmul, and ACT does
                        # nothing here so the next slice's first exp is not
                        # delayed behind eviction work
                        if tail:
                            nc.sync.dma_start(racc.ap(), acc[:])
                        u_sbs = []
                        for c in range(ND):
                            u_sb = usb_pool.tile(
                                [P, SL], F32, name=f"usb{c}", tag=f"usb{c}"
                            )
                            if tail and c % 2 == 1:
                                # no later exp to delay — ACT halves the tail
                                nc.scalar.copy(u_sb[:], us[c][:])
                            else:
                                nc.vector.tensor_copy(u_sb[:], us[c][:])
                            u_sbs.append(u_sb)
                        if tail:
                            # unnormalized exit; host divides at the gather
                            for c in range(ND):
                                store_eng = nc.sync if c % 2 == 0 else nc.scalar
                                store_eng.dma_start(
                                    ulast.ap()[c * P : (c + 1) * P, :], u_sbs[c][:]
                                )
                        else:
                            rall = misc.tile([P, SL], F32, name="rall", tag="rall")
                            nc.gpsimd.partition_all_reduce(
                                rall[:], acc[:], P, bass_isa.ReduceOp.add
                            )
                            rinv = misc.tile([P, SL], F32, name="rinv", tag="rinv")
                            if fast_recip:
                                nc.vector.reciprocal_approx_fast(rinv[:], rall[:])
                            else:
                                nc.vector.reciprocal(rinv[:], rall[:])
                            for c in range(ND):
                                nc.vector.tensor_tensor(
                                    u_sbs[c][:], u_sbs[c][:], rinv[:], op=ALU.mult
                                )
                                nc.sync.dma_start(
                                    out.ap()[c * P : (c + 1) * P, ns * SL : (ns + 1) * SL],
                                    u_sbs[c][:],
                                )

    nc.compile()
    return nc


_NC = None


def make_in_maps(x, W, b):
    x = np.ascontiguousarray(x, dtype=np.float32)
    W = np.asarray(W, dtype=np.float32)
    b = np.asarray(b, dtype=np.float32)
    Wq, Wk, Wv = W[:H], W[H : 2 * H], W[2 * H :]
    bq = b[:H]
    gm_host = np.ascontiguousarray(Wq.T @ Wk)                  # [768, 768]
    a = Wk.T @ bq                                              # [768]
    wvT = np.ascontiguousarray(Wv.T)                           # [768, 768]
    bcol = np.ascontiguousarray(a.reshape(ND, P).T)            # [128, 6]
    bvrep = np.ascontiguousarray(
        np.broadcast_to(b[2 * H :].reshape(1, H), (P, H))
    )
    return [
        {
            "xT": np.ascontiguousarray(x[i].T),                # [768, 2048]
            "wvT": wvT,
            "gm": gm_host,
            "bcol": bcol,
            "bvrep": bvrep,
        }
        for i in range(B)
    ]


def kernel(x: np.ndarray, W: np.ndarray, b: np.ndarray) -> np.ndarray:
    global _NC
    if _NC is None:
        _NC = build_nc()

    in_maps = make_in_maps(x, W, b)
    res = run_bass_kernel_spmd(_NC, in_maps, core_ids=list(range(B)))
    outs = []
    for i in range(B):
        oT = np.array(res.results[i]["out"])                   # [768, 2048]
        if "ulast" in res.results[i]:
            ul = np.asarray(res.results[i]["ulast"], dtype=np.float64)
            ra = np.asarray(res.results[i]["racc"], dtype=np.float64)
            r = ra.sum(axis=0)                                 # [512]
            lo = TAIL_SLICE * SL
            oT[:, lo : lo + SL] = (ul / r[None, :]).astype(np.float32)
        outs.append(np.ascontiguousarray(oT.T))
    return np.stack(outs, axis=0)
